# revision 11
# baseline (speedup 1.0000x reference)
"""AdaptivePCEN Trainium2 kernel.

Data-parallel over batch: core i computes batches [4i, 4i+4) of the
[32, 128, 4000] input. PPN weights replicated. Per core:
  - PE (bf16): h = relu(W1^T [Xprev; X] + b1), gates = W2^T h + b2,
    laid out so each gate lands as a [F=128, T_chunk] PSUM tile.
  - ACT: sigmoid/exp/ln gate evacuations + PCEN epilogue (no Softplus
    LUT on this toolchain: softplus = ln(1+exp(z))).
  - DVE: tensor_tensor_scan runs the EMA recurrence M_t = (1-s)M +
    s X along the free (time) axis, chained across chunks via the
    per-partition carry.
Matmul accumulation groups must stay inside one 2KB PSUM bank
(bank-crossing output corrupts), so chunks are 1024 cols with
bank-aligned 512-col sub-matmuls (ragged 928 tail).
"""

import numpy as np

B, F, T, H = 32, 128, 4000, 256
N_CORES = 8
BSH = B // N_CORES  # batches per core
CHA = 1024  # phase-A chunk (2 psum banks; subs at 0/512 bank-aligned)
SUBA = 512
PHB = 2  # phase-B pipeline chunks

_COMPILED = {}


def _chunks(t, ch):
    out = []
    t0 = 0
    while t0 < t:
        out.append((t0, min(ch, t - t0)))
        t0 += ch
    return out


def _build(bsh=BSH, t=T, cha=CHA, suba=SUBA, phb=PHB):
    from contextlib import ExitStack

    import concourse.tile as tile
    from concourse import bacc, mybir
    from concourse.tile_rust import add_dep_helper

    f32 = mybir.dt.float32
    bf16 = mybir.dt.bfloat16
    AF = mybir.ActivationFunctionType
    OP = mybir.AluOpType
    EPS = 1e-6

    nc = bacc.Bacc(
        "TRN2", target_bir_lowering=False, debug=False, num_devices=N_CORES
    )

    X = nc.dram_tensor("X", [bsh * F, t], f32, kind="ExternalInput").ap()
    W1 = nc.dram_tensor("W1", [2 * F, H], f32, kind="ExternalInput").ap()
    b1 = nc.dram_tensor("b1", [H, 1], f32, kind="ExternalInput").ap()
    W2 = nc.dram_tensor("W2", [H, 4 * F], f32, kind="ExternalInput").ap()
    b2 = nc.dram_tensor("b2", [4 * F, 1], f32, kind="ExternalInput").ap()
    out = nc.dram_tensor("out", [bsh * F, t], f32, kind="ExternalOutput").ap()

    assert t % phb == 0
    tb = t // phb

    with tile.TileContext(nc) as tc, ExitStack() as ctx:
        const = ctx.enter_context(tc.tile_pool(name="const", bufs=1))
        stag = ctx.enter_context(tc.tile_pool(name="stag", bufs=2))
        xpool = ctx.enter_context(tc.tile_pool(name="xpool", bufs=2))
        hpsum = ctx.enter_context(tc.tile_pool(name="hpsum", bufs=1, space="PSUM"))
        gpsum = ctx.enter_context(tc.tile_pool(name="gpsum", bufs=2, space="PSUM"))
        hsb = ctx.enter_context(tc.tile_pool(name="hsb", bufs=2))
        gates = ctx.enter_context(tc.tile_pool(name="gates", bufs=2))
        tmp = ctx.enter_context(tc.tile_pool(name="tmp", bufs=1))

        # ---- weights: DMA f32, cast to bf16 ----
        w1f = const.tile([F, 2 * H], f32, tag="w1f")
        nc.sync.dma_start(out=w1f[:, 0:H], in_=W1[0:F, :])
        nc.sync.dma_start(out=w1f[:, H : 2 * H], in_=W1[F : 2 * F, :])
        w1 = const.tile([F, 2 * H], bf16, tag="w1")
        nc.vector.tensor_copy(w1[:], w1f[:])
        w1a = w1[:, 0:H]  # W1 rows 0:F (Xprev part), [K=F, M=H]
        w1b = w1[:, H : 2 * H]  # W1 rows F:2F (X part)

        w2f = const.tile([F, 8 * F], f32, tag="w2f")
        nc.sync.dma_start(out=w2f[:, 0 : 4 * F], in_=W2[0:F, :])
        nc.sync.dma_start(out=w2f[:, 4 * F : 8 * F], in_=W2[F : 2 * F, :])
        w2 = const.tile([F, 8 * F], bf16, tag="w2")
        nc.vector.tensor_copy(w2[:], w2f[:])
        w2a = w2[:, 0 : 4 * F]  # W2 rows 0:H/2 (h1 part), [K, 4F]
        w2b = w2[:, 4 * F : 8 * F]  # W2 rows H/2:H (h2 part)

        bias1 = const.tile([F, 2], f32, tag="bias1")
        nc.sync.dma_start(out=bias1[:, 0:1], in_=b1[0:F, :])
        nc.sync.dma_start(out=bias1[:, 1:2], in_=b1[F : 2 * F, :])
        bias2 = const.tile([F, 4], f32, tag="bias2")
        for g in range(4):
            nc.sync.dma_start(
                out=bias2[:, g : g + 1], in_=b2[g * F : (g + 1) * F, :]
            )
        epsb = const.tile([F, 1], f32, tag="epsb")
        nc.vector.memset(epsb[:], EPS)

        prev_act = [None]  # last phase-B ACT inst of previous batch

        for b in range(bsh):
            # ---- load X[b], cast to bf16 with 2-col lead layout ----
            # xbuf col j (j>=2) = X[b,:,j-2]; col 1 = X[b,:,0] (X_prev edge)
            # Xcur view = xbuf[:, 2:t+2] (4B aligned), Xprev = xbuf[:, 1:t+1]
            xf = stag.tile([F, t], f32, tag="xf")
            nc.sync.dma_start(out=xf[:], in_=X[b * F : (b + 1) * F, :])
            xbuf = xpool.tile([F, t + 4], bf16, tag="xbuf")
            nc.vector.tensor_copy(xbuf[:, 2 : t + 2], xf[:])
            nc.vector.tensor_copy(xbuf[:, 1:2], xf[:, 0:1])
            xcur = xbuf[:, 2 : t + 2]

            s_sb = gates.tile([F, t], bf16, tag="s")
            al_sb = gates.tile([F, t], bf16, tag="al")
            r_sb = gates.tile([F, t], bf16, tag="r")
            zd_sb = gates.tile([F, t], bf16, tag="zd")

            first_sig = None
            last_sig = None

            for t0, cw in _chunks(t, cha):
                hp1 = hpsum.tile([F, cha], f32, tag="h1")
                hp2 = hpsum.tile([F, cha], f32, tag="h2")
                for s0, sw in _chunks(cw, suba):
                    xp = xbuf[:, 1 + t0 + s0 : 1 + t0 + s0 + sw]
                    xc = xbuf[:, 2 + t0 + s0 : 2 + t0 + s0 + sw]
                    sl = slice(s0, s0 + sw)
                    nc.tensor.matmul(hp1[:, sl], w1a[:, 0:F], xp,
                                     start=True, stop=False)
                    nc.tensor.matmul(hp1[:, sl], w1b[:, 0:F], xc,
                                     start=False, stop=True)
                    nc.tensor.matmul(hp2[:, sl], w1a[:, F:H], xp,
                                     start=True, stop=False)
                    nc.tensor.matmul(hp2[:, sl], w1b[:, F:H], xc,
                                     start=False, stop=True)
                h1s = hsb.tile([F, cha], bf16, tag="h1s")
                h2s = hsb.tile([F, cha], bf16, tag="h2s")
                nc.vector.tensor_scalar(
                    h1s[:, 0:cw], hp1[:, 0:cw], bias1[:, 0:1], 0.0,
                    OP.add, OP.max,
                )
                nc.vector.tensor_scalar(
                    h2s[:, 0:cw], hp2[:, 0:cw], bias1[:, 1:2], 0.0,
                    OP.add, OP.max,
                )

                # gates sequentially: s, alpha, r (ACT sigmoid), zd (DVE)
                for g, dest in ((0, s_sb), (1, al_sb), (3, r_sb), (2, zd_sb)):
                    gp = gpsum.tile([F, cha], f32, tag="g")
                    for s0, sw in _chunks(cw, suba):
                        sl = slice(s0, s0 + sw)
                        nc.tensor.matmul(
                            gp[:, sl], w2a[:, g * F : (g + 1) * F],
                            h1s[:, sl], start=True, stop=False,
                        )
                        nc.tensor.matmul(
                            gp[:, sl], w2b[:, g * F : (g + 1) * F],
                            h2s[:, sl], start=False, stop=True,
                        )
                    if g == 2:
                        nc.vector.tensor_scalar(
                            zd_sb[:, t0 : t0 + cw], gp[:, 0:cw],
                            bias2[:, 2:3], None, OP.add,
                        )
                    else:
                        inst = nc.scalar.activation(
                            dest[:, t0 : t0 + cw], gp[:, 0:cw], AF.Sigmoid,
                            bias=bias2[:, g : g + 1],
                        )
                        last_sig = inst
                        if first_sig is None:
                            first_sig = inst

            # ACT table grouping: this batch's sigmoids start only after the
            # previous batch's ln/exp epilogue is done.
            if prev_act[0] is not None:
                add_dep_helper(
                    first_sig.ins, prev_act[0].ins, sync=False,
                    reason="act table grouping",
                )

            # ---- phase B: chunked + pipelined over phb chunks ----
            # temps chain through 4 rotating slots (P,Q,R,S) per parity,
            # sized f32 [F, tb]; M{q} dedicated (its last column is the
            # scan carry read by the next chunk).
            carry = None
            for k in range(phb):
                q = k % 2

                names = iter(range(1000))

                def tl(slot, dt=bf16):
                    return tmp.tile(
                        [F, tb], dt, tag=f"{slot}{q}",
                        name=f"phb_{b}_{k}_{slot}{q}_{next(names)}",
                    )

                cs = slice(k * tb, (k + 1) * tb)
                xck = xbuf[:, 2 + k * tb : 2 + (k + 1) * tb]

                ez = tl("P")
                i_E = nc.scalar.activation(ez[:], zd_sb[:, cs], AF.Exp)
                # keep nl-set ops after this batch's sigmoid evacs
                add_dep_helper(i_E.ins, last_sig.ins, sync=False,
                               reason="sig before nl")
                dl = tl("Q")
                nc.scalar.activation(dl[:], ez[:], AF.Ln, bias=1.0)

                a_sb = tl("R")
                nc.vector.tensor_scalar(
                    a_sb[:], s_sb[:, cs], -1.0, 1.0, OP.mult, OP.add
                )
                bb = tl("S")
                nc.vector.tensor_tensor(bb[:], s_sb[:, cs], xck, OP.mult)

                M = tl("M", f32)
                nc.vector.tensor_tensor_scan(
                    M[:], a_sb[:], bb[:],
                    carry if carry is not None else 0.0,
                    OP.mult, OP.add,
                )
                carry = M[:, tb - 1 : tb]

                L = tl("R", f32)  # a freed after scan
                nc.scalar.activation(L[:], M[:], AF.Ln, bias=epsb[:])
                t1 = tl("S", f32)  # bb freed after scan
                nc.vector.tensor_tensor(t1[:], al_sb[:, cs], L[:], OP.mult)
                e1 = tl("P")  # ez freed after dl
                nc.scalar.activation(e1[:], t1[:], AF.Exp, scale=-1.0)
                num = tl("R")  # L freed after t1
                nc.vector.tensor_tensor(num[:], xck, e1[:], OP.mult)
                base = tl("S")  # t1 freed after e1
                nc.vector.tensor_tensor(base[:], num[:], dl[:], OP.add)
                lb = tl("P")  # e1 freed after num
                nc.scalar.activation(lb[:], base[:], AF.Ln)
                t2 = tl("R")  # num freed after base
                nc.vector.tensor_tensor(t2[:], r_sb[:, cs], lb[:], OP.mult)
                p1 = tl("S", f32)  # base freed after lb
                nc.scalar.activation(p1[:], t2[:], AF.Exp)

                ld = tl("P")  # lb freed after t2
                nc.scalar.activation(ld[:], dl[:], AF.Ln)
                t3 = tl("R")  # t2 freed after p1
                nc.vector.tensor_tensor(t3[:], r_sb[:, cs], ld[:], OP.mult)
                p2 = tl("Q", f32)  # dl freed after ld
                i_p2 = nc.scalar.activation(p2[:], t3[:], AF.Exp)
                prev_act[0] = i_p2

                ob = tl("P", f32)  # ld freed after t3
                nc.vector.tensor_tensor(ob[:], p1[:], p2[:], OP.subtract)
                nc.sync.dma_start(
                    out=out[b * F : (b + 1) * F, cs], in_=ob[:]
                )

    nc.compile()
    return nc


def _get(key=(BSH, T, CHA, SUBA, PHB)):
    if key not in _COMPILED:
        _COMPILED[key] = _build(*key)
    return _COMPILED[key]


def _in_maps(X, W1, b1, W2, b2):
    maps = []
    for i in range(N_CORES):
        maps.append(
            {
                "X": np.ascontiguousarray(
                    X[i * BSH : (i + 1) * BSH].reshape(BSH * F, T)
                ),
                "W1": np.ascontiguousarray(W1),
                "b1": np.ascontiguousarray(b1.reshape(H, 1)),
                "W2": np.ascontiguousarray(W2),
                "b2": np.ascontiguousarray(b2.reshape(4 * F, 1)),
            }
        )
    return maps


def run(X, W1, b1, W2, b2, trace=False, **kw):
    from concourse.bass_utils import run_bass_kernel_spmd

    nc = _get()
    res = run_bass_kernel_spmd(
        nc,
        _in_maps(X, W1, b1, W2, b2),
        core_ids=list(range(N_CORES)),
        trace=trace,
        **kw,
    )
    out = np.concatenate(
        [res.results[i]["out"].reshape(BSH, F, T) for i in range(N_CORES)],
        axis=0,
    ).astype(np.float32)
    return out, res


def kernel(X, W1, b1, W2, b2):
    return run(X, W1, b1, W2, b2)[0]


# revision 16
# speedup vs baseline: 1.3705x; 1.3705x over previous
"""AdaptivePCEN Trainium2 kernel.

Data-parallel over batch: core i computes batches [4i, 4i+4) of the
[32, 128, 4000] input. PPN weights replicated. Per core:
  - PE (bf16): h = relu(W1^T [Xprev; X] + b1), gates = W2^T h + b2,
    laid out so each gate lands as a [F=128, T_chunk] PSUM tile.
  - ACT: sigmoid/exp/ln gate evacuations + PCEN epilogue (no Softplus
    LUT on this toolchain: softplus = ln(1+exp(z))).
  - DVE: tensor_tensor_scan runs the EMA recurrence M_t = (1-s)M +
    s X along the free (time) axis, chained across chunks via the
    per-partition carry.
Matmul accumulation groups must stay inside one 2KB PSUM bank
(bank-crossing output corrupts), so chunks are 1024 cols with
bank-aligned 512-col sub-matmuls (ragged 928 tail).
"""

import numpy as np

B, F, T, H = 32, 128, 4000, 256
N_CORES = 8
BSH = B // N_CORES  # batches per core
CHA = 1024  # phase-A chunk (2 psum banks; subs at 0/512 bank-aligned)
SUBA = 512
PHB = 2  # phase-B pipeline chunks

_COMPILED = {}


def _chunks(t, ch):
    out = []
    t0 = 0
    while t0 < t:
        out.append((t0, min(ch, t - t0)))
        t0 += ch
    return out


def _build(bsh=BSH, t=T, cha=CHA, suba=SUBA, phb=PHB):
    from contextlib import ExitStack

    import concourse.tile as tile
    from concourse import bacc, mybir
    from concourse.tile_rust import add_dep_helper

    f32 = mybir.dt.float32
    bf16 = mybir.dt.bfloat16
    AF = mybir.ActivationFunctionType
    OP = mybir.AluOpType
    EPS = 1e-6

    nc = bacc.Bacc(
        "TRN2", target_bir_lowering=False, debug=False, num_devices=N_CORES
    )

    X = nc.dram_tensor("X", [bsh * F, t], f32, kind="ExternalInput").ap()
    W1 = nc.dram_tensor("W1", [2 * F, H], f32, kind="ExternalInput").ap()
    b1 = nc.dram_tensor("b1", [H, 1], f32, kind="ExternalInput").ap()
    W2 = nc.dram_tensor("W2", [H, 4 * F], f32, kind="ExternalInput").ap()
    b2 = nc.dram_tensor("b2", [4 * F, 1], f32, kind="ExternalInput").ap()
    out = nc.dram_tensor("out", [bsh * F, t], f32, kind="ExternalOutput").ap()

    assert t % phb == 0
    tb = t // phb

    with tile.TileContext(nc) as tc, ExitStack() as ctx:
        const = ctx.enter_context(tc.tile_pool(name="const", bufs=1))
        stag = ctx.enter_context(tc.tile_pool(name="stag", bufs=2))
        xpool = ctx.enter_context(tc.tile_pool(name="xpool", bufs=2))
        hpsum = ctx.enter_context(tc.tile_pool(name="hpsum", bufs=1, space="PSUM"))
        gpsum = ctx.enter_context(tc.tile_pool(name="gpsum", bufs=2, space="PSUM"))
        hsb = ctx.enter_context(tc.tile_pool(name="hsb", bufs=2))
        gates = ctx.enter_context(tc.tile_pool(name="gates", bufs=2))
        tmp = ctx.enter_context(tc.tile_pool(name="tmp", bufs=1))

        # ---- weights: DMA f32, cast to bf16 ----
        w1f = const.tile([F, 2 * H], f32, tag="w1f")
        nc.sync.dma_start(out=w1f[:, 0:H], in_=W1[0:F, :])
        nc.sync.dma_start(out=w1f[:, H : 2 * H], in_=W1[F : 2 * F, :])
        w1 = const.tile([F, 2 * H], bf16, tag="w1")
        nc.vector.tensor_copy(w1[:], w1f[:])
        w1a = w1[:, 0:H]  # W1 rows 0:F (Xprev part), [K=F, M=H]
        w1b = w1[:, H : 2 * H]  # W1 rows F:2F (X part)

        w2f = const.tile([F, 8 * F], f32, tag="w2f")
        nc.sync.dma_start(out=w2f[:, 0 : 4 * F], in_=W2[0:F, :])
        nc.sync.dma_start(out=w2f[:, 4 * F : 8 * F], in_=W2[F : 2 * F, :])
        w2 = const.tile([F, 8 * F], bf16, tag="w2")
        nc.vector.tensor_copy(w2[:], w2f[:])
        w2a = w2[:, 0 : 4 * F]  # W2 rows 0:H/2 (h1 part), [K, 4F]
        w2b = w2[:, 4 * F : 8 * F]  # W2 rows H/2:H (h2 part)

        bias1 = const.tile([F, 2], f32, tag="bias1")
        nc.sync.dma_start(out=bias1[:, 0:1], in_=b1[0:F, :])
        nc.sync.dma_start(out=bias1[:, 1:2], in_=b1[F : 2 * F, :])
        bias2 = const.tile([F, 4], f32, tag="bias2")
        for g in range(4):
            nc.sync.dma_start(
                out=bias2[:, g : g + 1], in_=b2[g * F : (g + 1) * F, :]
            )
        epsb = const.tile([F, 1], f32, tag="epsb")
        nc.vector.memset(epsb[:], EPS)

        prev_act = [None]  # last ACT inst of previous batch's chain
        NL_SET = 6  # natural_log_exp_and_others in act_info.json

        for b in range(bsh):
            # ---- load X[b], cast to bf16 with 2-col lead layout ----
            # xbuf col j (j>=2) = X[b,:,j-2]; col 1 = X[b,:,0] (X_prev edge)
            # Xcur view = xbuf[:, 2:t+2] (4B aligned), Xprev = xbuf[:, 1:t+1]
            xf = stag.tile([F, t], f32, tag="xf")
            nc.sync.dma_start(out=xf[:], in_=X[b * F : (b + 1) * F, :])
            xbuf = xpool.tile([F, t + 4], bf16, tag="xbuf")
            nc.vector.tensor_copy(xbuf[:, 2 : t + 2], xf[:])
            nc.vector.tensor_copy(xbuf[:, 1:2], xf[:, 0:1])
            xcur = xbuf[:, 2 : t + 2]

            s_sb = gates.tile([F, t], bf16, tag="s")
            al_sb = gates.tile([F, t], bf16, tag="al")
            r_sb = gates.tile([F, t], bf16, tag="r")
            zd_sb = gates.tile([F, t], bf16, tag="zd")

            sig_insts = []

            for t0, cw in _chunks(t, cha):
                hp1 = hpsum.tile([F, cha], f32, tag="h1")
                hp2 = hpsum.tile([F, cha], f32, tag="h2")
                for s0, sw in _chunks(cw, suba):
                    xp = xbuf[:, 1 + t0 + s0 : 1 + t0 + s0 + sw]
                    xc = xbuf[:, 2 + t0 + s0 : 2 + t0 + s0 + sw]
                    sl = slice(s0, s0 + sw)
                    nc.tensor.matmul(hp1[:, sl], w1a[:, 0:F], xp,
                                     start=True, stop=False)
                    nc.tensor.matmul(hp1[:, sl], w1b[:, 0:F], xc,
                                     start=False, stop=True)
                    nc.tensor.matmul(hp2[:, sl], w1a[:, F:H], xp,
                                     start=True, stop=False)
                    nc.tensor.matmul(hp2[:, sl], w1b[:, F:H], xc,
                                     start=False, stop=True)
                h1s = hsb.tile([F, cha], bf16, tag="h1s")
                h2s = hsb.tile([F, cha], bf16, tag="h2s")
                nc.vector.tensor_scalar(
                    h1s[:, 0:cw], hp1[:, 0:cw], bias1[:, 0:1], 0.0,
                    OP.add, OP.max,
                )
                nc.vector.tensor_scalar(
                    h2s[:, 0:cw], hp2[:, 0:cw], bias1[:, 1:2], 0.0,
                    OP.add, OP.max,
                )

                # gates sequentially: s, alpha, r (ACT sigmoid), zd (DVE)
                for g, dest in ((0, s_sb), (1, al_sb), (3, r_sb), (2, zd_sb)):
                    gp = gpsum.tile([F, cha], f32, tag="g")
                    for s0, sw in _chunks(cw, suba):
                        sl = slice(s0, s0 + sw)
                        nc.tensor.matmul(
                            gp[:, sl], w2a[:, g * F : (g + 1) * F],
                            h1s[:, sl], start=True, stop=False,
                        )
                        nc.tensor.matmul(
                            gp[:, sl], w2b[:, g * F : (g + 1) * F],
                            h2s[:, sl], start=False, stop=True,
                        )
                    if g == 2:
                        nc.vector.tensor_scalar(
                            zd_sb[:, t0 : t0 + cw], gp[:, 0:cw],
                            bias2[:, 2:3], None, OP.add,
                        )
                    else:
                        sig_insts.append(
                            nc.scalar.activation(
                                dest[:, t0 : t0 + cw], gp[:, 0:cw], AF.Sigmoid,
                                bias=bias2[:, g : g + 1],
                            )
                        )

            # Total ACT order per batch (scheduler ignores sync=False hints):
            # [sigmoids] -> LoadActFuncSet(nl_exp) -> grouped ln/exp epilogue.
            # Keeps table loads at 2/batch instead of one per ln<->exp flip.
            act_chain = list(sig_insts)
            ld_inst = nc.scalar.add_instruction(
                mybir.InstLoadActFuncSet(
                    name=nc.get_next_instruction_name(),
                    act_func_set_id=NL_SET,
                    ins=[],
                    outs=[],
                )
            )
            act_chain.append(ld_inst)

            # ---- phase B ----
            # delta path is full-tensor (not in the scan pipeline): E=exp(zd),
            # delta=ln(1+E), ld=ln(delta). F1 holds E then ld; F2 holds delta.
            ez = tmp.tile([F, t], bf16, tag="F1", name=f"ez_{b}")
            i_E = nc.scalar.activation(ez[:], zd_sb[:], AF.Exp)
            dl = tmp.tile([F, t], bf16, tag="F2", name=f"dl_{b}")
            i_dl = nc.scalar.activation(dl[:], ez[:], AF.Ln, bias=1.0)
            ld = tmp.tile([F, t], bf16, tag="F1", name=f"ld_{b}")
            i_ld = nc.scalar.activation(ld[:], dl[:], AF.Ln)

            # scan + PCEN epilogue pipelined over phb chunks; temps rotate
            # through slots P,R,S per parity; M{q} dedicated (its last
            # column is the scan carry read by the next chunk).
            chunk_insts = []
            carry = None
            for k in range(phb):
                q = k % 2

                names = iter(range(1000))

                def tl(slot, dt=bf16):
                    return tmp.tile(
                        [F, tb], dt, tag=f"{slot}{q}",
                        name=f"phb_{b}_{k}_{slot}{q}_{next(names)}",
                    )

                cs = slice(k * tb, (k + 1) * tb)
                xck = xbuf[:, 2 + k * tb : 2 + (k + 1) * tb]

                a_sb = tl("R")
                nc.vector.tensor_scalar(
                    a_sb[:], s_sb[:, cs], -1.0, 1.0, OP.mult, OP.add
                )
                bb = tl("S")
                nc.vector.tensor_tensor(bb[:], s_sb[:, cs], xck, OP.mult)

                M = tl("M", f32)
                nc.vector.tensor_tensor_scan(
                    M[:], a_sb[:], bb[:],
                    carry if carry is not None else 0.0,
                    OP.mult, OP.add,
                )
                carry = M[:, tb - 1 : tb]

                L = tl("R", f32)  # a freed after scan
                i_L = nc.scalar.activation(L[:], M[:], AF.Ln, bias=epsb[:])
                t1 = tl("S", f32)  # bb freed after scan
                nc.vector.tensor_tensor(t1[:], al_sb[:, cs], L[:], OP.mult)
                e1 = tl("P")
                i_e1 = nc.scalar.activation(e1[:], t1[:], AF.Exp, scale=-1.0)
                num = tl("R")  # L freed after t1
                nc.vector.tensor_tensor(num[:], xck, e1[:], OP.mult)
                base = tl("S")  # t1 freed after e1
                nc.vector.tensor_tensor(base[:], num[:], dl[:, cs], OP.add)
                lb = tl("P")  # e1 freed after num
                i_lb = nc.scalar.activation(lb[:], base[:], AF.Ln)
                t2 = tl("R")  # num freed after base
                nc.vector.tensor_tensor(t2[:], r_sb[:, cs], lb[:], OP.mult)
                p1 = tl("S", f32)  # base freed after lb
                i_p1 = nc.scalar.activation(p1[:], t2[:], AF.Exp)
                t3 = tl("R")  # t2 freed after p1
                nc.vector.tensor_tensor(t3[:], r_sb[:, cs], ld[:, cs], OP.mult)
                p2 = tl("P", f32)  # lb freed after t2
                i_p2 = nc.scalar.activation(p2[:], t3[:], AF.Exp)

                ob = tl("R", f32)  # t3 freed after p2
                nc.vector.tensor_tensor(ob[:], p1[:], p2[:], OP.subtract)
                nc.sync.dma_start(
                    out=out[b * F : (b + 1) * F, cs], in_=ob[:]
                )
                chunk_insts.append((i_L, i_e1, i_lb, i_p1, i_p2))

            # ACT chain: E, dl, ld early; then per-func pairs across chunks
            # so the scheduler can't flip ln/exp mid-stream.
            act_chain.extend([i_E, i_dl, i_ld])
            for idx in range(5):
                for k in range(phb):
                    act_chain.append(chunk_insts[k][idx])
            if prev_act[0] is not None:
                add_dep_helper(
                    act_chain[0].ins, prev_act[0].ins, sync=True,
                    reason="batch act order",
                )
            for prv, nxt in zip(act_chain, act_chain[1:]):
                add_dep_helper(nxt.ins, prv.ins, sync=True, reason="act order")
            prev_act[0] = act_chain[-1]

    nc.compile()
    return nc


def _get(key=(BSH, T, CHA, SUBA, PHB)):
    if key not in _COMPILED:
        _COMPILED[key] = _build(*key)
    return _COMPILED[key]


def _in_maps(X, W1, b1, W2, b2):
    maps = []
    for i in range(N_CORES):
        maps.append(
            {
                "X": np.ascontiguousarray(
                    X[i * BSH : (i + 1) * BSH].reshape(BSH * F, T)
                ),
                "W1": np.ascontiguousarray(W1),
                "b1": np.ascontiguousarray(b1.reshape(H, 1)),
                "W2": np.ascontiguousarray(W2),
                "b2": np.ascontiguousarray(b2.reshape(4 * F, 1)),
            }
        )
    return maps


def run(X, W1, b1, W2, b2, trace=False, **kw):
    from concourse.bass_utils import run_bass_kernel_spmd

    nc = _get()
    res = run_bass_kernel_spmd(
        nc,
        _in_maps(X, W1, b1, W2, b2),
        core_ids=list(range(N_CORES)),
        trace=trace,
        **kw,
    )
    out = np.concatenate(
        [res.results[i]["out"].reshape(BSH, F, T) for i in range(N_CORES)],
        axis=0,
    ).astype(np.float32)
    return out, res


def kernel(X, W1, b1, W2, b2):
    return run(X, W1, b1, W2, b2)[0]
